# revision 63
# baseline (speedup 1.0000x reference)
"""Span-attention kernel for Trainium2 (8 NeuronCores, SPMD).

Strategy (v4)
-------------
Data-parallel over bsz: core b owns batch row b (bsz == 8 == n_cores).
Host routes each query q to core qb[q]; both span sets are pooled
(the mask depends only on (start, end)) and bucketed by start>>7.
Each of the 16 buckets gets ONE primary query tile (128 slots) with a
2-chunk token window; per-core overflow goes to ovfA tiles (starts in
[0,1024), window chunks 0..8) and ovfB tiles (starts in [1024,2048),
window chunks 8..15).

Device traffic is minimized and DMA-friendly:
  * x and wext are pre-tiled on the host into the exact SBUF layout
    ([128 part, 128x128 k-tiles], bf16), one fully-contiguous DRAM
    param per load DMA.
  * masks are host-built dense fp8 {0,1} tiles in [token, query] lhsT
    layout -- no on-device mask construction.
  * output is the unnormalized [num | den] (bf16); the host divides.

Schedule: 12 dummy matmuls on a zero tile bridge the DMA ramp so the
PE HAM clock is un-throttled when real work starts; W is split across
both HWDGE rings ahead of the x slabs; the x feed runs on the sync
ring in consumption order; masks stream on the gpsimd ring in two
pieces (the late piece gated behind the x feed).  enc chunks are
produced in rotated order [1..15, 0] and phase-2 matmuls trail RUNWAY
iterations behind so mask-waiting work never head-of-line blocks the
PE queue; only primary 0 + ovfA remain after the last enc chunk.
Results stage as 4 three-tile groups (scalar-ring DMAs) + 6 singles
with dedicated staging tiles, the last finishers fanned across all
three rings.

Per-core device program:
  1. enc[2048, 257] = X_b @ [W | W@attn_w]  (PE bf16, 8 k-tiles per
     128-token chunk); ACT: E = exp(logit+bias); DVE scales the 256
     value cols by E into bf16 EncE; ACT writes E into col 256.
  2. out_ps[q, 0:257] = sum_w mask[w]^T @ EncE[chunk(w)]  (PE, fp8
     lhsT x bf16 rhs); overflow tiles accumulate one chunk per
     iteration so the tail is one matmul deep.
  3. DVE/ACT copy out_ps -> bf16 staging -> DMA out.
"""

import os
import sys

import numpy as np
import ml_dtypes

sys.path.insert(0, "/opt/trn_rl_repo")

from contextlib import ExitStack

from concourse import bass, bacc, mybir
import concourse.tile as tile
from concourse.bass_utils import run_bass_kernel_spmd

P = 128
BSZ = 8
SEQ = 2048
HD = 1024
PD = 256
NCOL = PD + 1   # value cols + logit col
NOUT = PD + 1   # value cols + denominator col
NB = SEQ // P   # 16 buckets
KT = HD // P    # 8 contraction tiles
Q = 8192
WCOLS = KT * NCOL          # 2056 pre-tiled wext columns
XOFF = WCOLS               # x tiles start here in the fused buffer
NA_W = 9                   # ovfA window chunks (0..8)
NB_W = 8                   # ovfB window chunks (8..15)

_cache = {}


_SIGMA = list(range(1, NB)) + [0]   # enc chunk production order


def _finish_order(FA, FB):
    """Tile ids in device finish order (res slot s holds the s-th
    finisher).  Rotated production [1..15, 0] means primary 0 and ovfA
    (whose windows contain chunk 0) finish last."""
    order = list(range(1, NB - 2))                    # primaries 1..13
    order += [NB + FA + b for b in range(FB)]         # ovfB (stop chunk 15)
    order += [NB - 2, NB - 1]                         # primaries 14, 15
    order += [NB + a for a in range(FA)]              # ovfA (stop chunk 0)
    order += [0]                                      # primary 0
    return order


def _mask_chunk(i, w, FA):
    """Chunk index in maskbuf for (tile i, window-pos w).
    Order: primaries 0..7, ovfA tiles, primaries 8..15, ovfB tiles."""
    if i < NB // 2:
        return i * 2 + w
    if NB <= i < NB + FA:
        return NB + (i - NB) * NA_W + w
    if i < NB:
        return NB + FA * NA_W + (i - NB // 2) * 2 + w
    return NB + FA * NA_W + NB + (i - NB - FA) * NB_W + w


def _build_program(FA, FB, logit_bias=0.0):
    T = NB + FA + FB
    NCH = NB * 2 + FA * NA_W + FB * NB_W
    NG = (T + 2) // 3
    nc = bacc.Bacc("TRN2", target_bir_lowering=False)
    f32 = mybir.dt.float32
    bf16 = mybir.dt.bfloat16
    fp8 = mybir.dt.float8e4

    # separate params per load slab: each DMA reads a fully-contiguous
    # DRAM region (strided column slices of one big buffer measured only
    # ~240 GB/s)
    w0a = nc.declare_dram_parameter("w0a", [P, WCOLS // 2], bf16,
                                    isOutput=False)
    w0b = nc.declare_dram_parameter("w0b", [P, WCOLS // 2], bf16,
                                    isOutput=False)
    xslabs = [nc.declare_dram_parameter(f"xs{j}", [P, KT * P], bf16,
                                        isOutput=False) for j in range(NB)]
    maskbuf = nc.declare_dram_parameter("maskbuf", [P, NCH * P], fp8,
                                        isOutput=False)
    # res groups: threes for the bulk (own param: contiguous 1.5 KiB
    # descriptor runs), singles for the last six finishers so the tail
    # drains in parallel across rings
    NS = 6
    groups = []
    for a in range(0, T - NS, 3):
        groups.append((a, min(a + 3, T - NS)))
    N3 = len(groups)
    groups += [(T - NS + s, T - NS + s + 1) for s in range(NS)]
    tile2group = {}
    for gi, (a, b) in enumerate(groups):
        for s in range(a, b):
            tile2group[s] = gi
    res3 = nc.declare_dram_parameter("res3", [N3, P, 3 * NOUT], bf16,
                                     isOutput=True)
    res1 = nc.declare_dram_parameter("res1", [NS, P, NOUT], bf16,
                                     isOutput=True)

    def mcol(i, w):
        return _mask_chunk(i, w, FA) * P

    MASK_SPLIT = (NB + FA * NA_W) * P   # end of the early piece

    with tile.TileContext(nc) as tc, ExitStack() as ctx:
        xw_pool = ctx.enter_context(tc.tile_pool(name="xw", bufs=1))
        mask_pool = ctx.enter_context(tc.tile_pool(name="mask", bufs=1))
        ecol_pool = ctx.enter_context(tc.tile_pool(name="ecol", bufs=1))
        ence_pool = ctx.enter_context(tc.tile_pool(name="ence", bufs=1))
        out_pool = ctx.enter_context(tc.tile_pool(name="out", bufs=3))
        single_pool = ctx.enter_context(tc.tile_pool(name="single", bufs=1))
        # PSUM budget: enc (shared with warmup) + out + (FA+FB ovf) <= 8
        spare = max(0, FA + FB - 2)
        ps_enc = ctx.enter_context(tc.tile_pool(name="ps_enc", bufs=max(2, 3 - spare), space="PSUM"))
        ps_out = ctx.enter_context(tc.tile_pool(name="ps_out", bufs=3, space="PSUM"))
        ps_ovf = ctx.enter_context(tc.tile_pool(name="ps_ovf", bufs=1, space="PSUM"))

        # ---- PE warmup: dummy matmuls bridge the DMA ramp so HAM is
        # un-throttled before the first real matmul ----
        warm_pool = ctx.enter_context(tc.tile_pool(name="warm", bufs=1))
        warm_sb = warm_pool.tile([P, 512], bf16, tag="warm_sb")
        nc.gpsimd.memset(warm_sb[:], 0.0)
        warm_ps = ps_enc.tile([P, 512], f32, tag="enc")
        for _ in range(12):
            nc.tensor.matmul(warm_ps[:], lhsT=warm_sb[:, 0:P],
                             rhs=warm_sb[:], start=True, stop=True,
                             skip_group_check=True)
        # keep the warmup alive past DCE: its result feeds a dram output
        warm_out = nc.declare_dram_parameter("warm_out", [1, 1], f32,
                                             isOutput=True)
        warm_res = warm_pool.tile([1, 1], f32, tag="warm_res")
        nc.vector.tensor_copy(warm_res[:], warm_ps[0:1, 0:1])
        nc.scalar.dma_start(warm_out[:], warm_res[:])

        # ---- loads: w on the scalar ring in parallel with the x feed on
        # the sync ring; masks on the gpsimd ring ----
        xw_sb = xw_pool.tile([P, WCOLS + NB * KT * P], bf16, tag="xw_sb")
        mask_sb = mask_pool.tile([P, NCH * P], fp8, tag="mask_sb")
        # W split across both HWDGE rings: k0-3 first on sync (ahead of the
        # x slabs), k4-7 on scalar -- the m0 k<4 matmuls can start as soon
        # as w0a+m0 land
        wa = nc.sync.dma_start(xw_sb[:, 0:WCOLS // 2], w0a[:])
        nc.scalar.dma_start(xw_sb[:, WCOLS // 2:WCOLS], w0b[:])
        nc.gpsimd.dma_start(mask_sb[:, 0:MASK_SPLIT], maskbuf[:, 0:MASK_SPLIT])
        slab_dmas = []
        for j in range(NB):
            c0 = WCOLS + j * KT * P
            # slab 1 rides the scalar ring right behind w0b so the PE never
            # starves between m0 and m1
            eng = nc.scalar if j == 1 else nc.sync
            slab_dmas.append(
                eng.dma_start(xw_sb[:, c0:c0 + KT * P], xslabs[j][:]))
        m2 = nc.gpsimd.dma_start(mask_sb[:, MASK_SPLIT:NCH * P],
                                 maskbuf[:, MASK_SPLIT:NCH * P])
        # late mask piece only needed from primary-8 emit; let x stream first
        bass._add_dep_helper(m2.ins, slab_dmas[6].ins, sync=True,
                             reason="stage late masks behind x feed")

        w_tiles = [xw_sb[:, k * NCOL:(k + 1) * NCOL] for k in range(KT)]
        enc_tiles = [None] * NB
        ovfA_ps = [None] * FA
        ovfB_ps = [None] * FB
        res_group = {}   # g -> [staging tile, n_written]
        slot_ctr = [0]

        def finish_tile(out_ps):
            slot = slot_ctr[0]
            slot_ctr[0] += 1
            g = tile2group[slot]
            a, bnd = groups[g]
            h = slot - a
            if g not in res_group:
                if g < N3:
                    rt = out_pool.tile([P, (bnd - a) * NOUT], bf16, tag="res")
                else:
                    # dedicated staging per single: the tail must never wait
                    # on an earlier group's DMA to free a pool slot
                    rt = single_pool.tile([P, NOUT], bf16, tag=f"ress{g - N3}")
                res_group[g] = [rt, 0]
            rg = res_group[g]
            use_dve = slot == T - 1 if slot >= T - 2 else slot % 2 == 0
            if use_dve:
                nc.vector.tensor_copy(rg[0][:, h * NOUT:(h + 1) * NOUT],
                                      out_ps[:])
            else:
                nc.scalar.activation(rg[0][:, h * NOUT:(h + 1) * NOUT],
                                     out_ps[:],
                                     mybir.ActivationFunctionType.Copy)
            rg[1] += 1
            if rg[1] == bnd - a:
                if g < N3:
                    n = bnd - a
                    nc.scalar.dma_start(res3[g][:, 0:n * NOUT], rg[0][:])
                else:
                    eng = (nc.scalar, nc.sync, nc.gpsimd,
                           nc.scalar, nc.gpsimd, nc.sync)[g - N3]
                    eng.dma_start(res1[a - (T - NS)], rg[0][:])

        def emit_primary(i):
            cs = [min(i, NB - 2), min(i, NB - 2) + 1]
            out_ps = ps_out.tile([P, NOUT], f32, tag="out")
            for w, c in enumerate(cs):
                nc.tensor.matmul(out_ps[:],
                                 lhsT=mask_sb[:, mcol(i, w):mcol(i, w) + P],
                                 rhs=enc_tiles[c][:],
                                 start=(w == 0), stop=(w == 1))
            finish_tile(out_ps)

        # phase-2 work for "virtual iteration" m, delayed RUNWAY iterations
        # behind enc production so mask-waiting matmuls never head-of-line
        # block the PE queue while the mask DMA is still in flight
        RUNWAY = 3

        def step(t):
            tpos = t - RUNWAY
            if not 0 <= tpos <= NB - 1:
                return
            c = _SIGMA[tpos]
            if c <= NA_W - 1:
                # ovfA accumulates in arrival order: chunk 1 first (tpos 0),
                # chunk 0 last (tpos NB-1)
                for a in range(FA):
                    if tpos == 0:
                        ova_tile = ps_ovf.tile([P, NOUT], f32, tag=f"ovA{a}")
                        ovfA_ps[a] = ova_tile
                    nc.tensor.matmul(
                        ovfA_ps[a][:],
                        lhsT=mask_sb[:, mcol(NB + a, c):mcol(NB + a, c) + P],
                        rhs=enc_tiles[c][:], start=(tpos == 0),
                        stop=(tpos == NB - 1), skip_group_check=True)
            if NB - NB_W <= c <= NB - 1:
                for b in range(FB):
                    w = c - (NB - NB_W)
                    if w == 0:
                        ovb_tile = ps_ovf.tile([P, NOUT], f32, tag=f"ovB{b}")
                        ovfB_ps[b] = ovb_tile
                    nc.tensor.matmul(
                        ovfB_ps[b][:],
                        lhsT=mask_sb[:, mcol(NB + FA + b, w):mcol(NB + FA + b, w) + P],
                        rhs=enc_tiles[c][:], start=(w == 0),
                        stop=(w == NB_W - 1), skip_group_check=True)
            # finish order must match _finish_order()
            if tpos == NB - 2:
                for b in range(FB):
                    finish_tile(ovfB_ps[b])
            if 1 <= tpos <= NB - 3:
                emit_primary(tpos)
            if tpos == NB - 2:
                emit_primary(NB - 2)
                emit_primary(NB - 1)
            if tpos == NB - 1:
                for a in range(FA):
                    finish_tile(ovfA_ps[a])
                emit_primary(0)

        # ---- phase 1 with interleaved (delayed) phase 2, enc chunks
        # produced in _SIGMA order ----
        for t in range(NB):
            m = _SIGMA[t]
            enc_ps = ps_enc.tile([P, NCOL], f32, tag="enc")
            xbase = XOFF + t * KT * P
            for k in range(KT):
                nc.tensor.matmul(
                    enc_ps[:], lhsT=xw_sb[:, xbase + k * P:xbase + (k + 1) * P],
                    rhs=w_tiles[k], start=(k == 0), stop=(k == KT - 1))
            ecol = ecol_pool.tile([P, 1], f32, tag=f"ecol{m}")
            nc.scalar.activation(ecol[:], enc_ps[:, PD:PD + 1],
                                 mybir.ActivationFunctionType.Exp,
                                 bias=float(logit_bias))
            ence = ence_pool.tile([P, NOUT], bf16, tag=f"ence{m}")
            nc.vector.tensor_scalar_mul(ence[:, 0:PD], enc_ps[:, 0:PD], ecol[:])
            nc.scalar.activation(ence[:, PD:PD + 1], ecol[:],
                                 mybir.ActivationFunctionType.Copy)
            enc_tiles[m] = ence
            step(t)
        for t in range(NB, NB + RUNWAY):
            step(t)

    nc.compile()
    return nc


def _prep(inputs):
    enc_in = np.asarray(inputs["encoded_input"], np.float32)
    proj_w = np.asarray(inputs["proj_w"], np.float32)
    proj_b = np.asarray(inputs["proj_b"], np.float32)
    attn_w = np.asarray(inputs["attn_w"], np.float32)
    attn_b = np.float32(np.asarray(inputs["attn_b"], np.float32))
    qb = np.asarray(inputs["query_batch_idx"], np.int64)
    s_all = [np.asarray(inputs["start_ids_1"], np.int64),
             np.asarray(inputs["start_ids_2"], np.int64)]
    e_all = [np.asarray(inputs["end_ids_1"], np.int64),
             np.asarray(inputs["end_ids_2"], np.int64)]

    waw = (proj_w @ attn_w)[:, None]
    wext = np.concatenate([proj_w, waw], axis=1)          # [HD, 257]
    logit_bias = float(proj_b @ attn_w + attn_b)
    use_bias = bool(np.any(proj_b != 0.0))
    wtiled = wext.reshape(KT, P, NCOL).transpose(1, 0, 2).reshape(P, WCOLS)

    tok = np.arange(P)
    # ---- bucket queries per core ----
    core_data = []
    FA = FB = 1
    for b in range(BSZ):
        sel = np.nonzero(qb == b)[0]
        prim = {kb: ([], [], []) for kb in range(NB)}
        oa_s, oa_e, oa_sc = [], [], []
        ob_s, ob_e, ob_sc = [], [], []
        for ss in range(2):
            s = s_all[ss][sel]
            e = e_all[ss][sel]
            kk = (s >> 7).astype(np.int64)
            for kb in range(NB):
                g = np.nonzero(kk == kb)[0]
                cur = prim[kb]
                room = P - len(cur[0])
                take, rest = g[:room], g[room:]
                cur[0].extend(s[take])
                cur[1].extend(e[take])
                cur[2].extend((ss, qi) for qi in sel[take])
                if len(rest):
                    if kb < NB // 2:
                        oa_s.extend(s[rest]); oa_e.extend(e[rest])
                        oa_sc.extend((ss, qi) for qi in sel[rest])
                    else:
                        ob_s.extend(s[rest]); ob_e.extend(e[rest])
                        ob_sc.extend((ss, qi) for qi in sel[rest])
        core_data.append((prim, (oa_s, oa_e, oa_sc), (ob_s, ob_e, ob_sc)))
        FA = max(FA, (len(oa_s) + P - 1) // P)
        FB = max(FB, (len(ob_s) + P - 1) // P)

    T = NB + FA + FB
    NCH = NB * 2 + FA * NA_W + FB * NB_W
    NG = (T + 2) // 3

    def fill(maskbuf, col0, nw, crow0, ss, ee):
        n = len(ss)
        if not n:
            return
        sa, ea = np.asarray(ss), np.asarray(ee)
        for w in range(nw):
            rows = tok + (crow0 + w) * P
            m = (rows[:, None] >= sa[None, :]) & (rows[:, None] <= ea[None, :])
            maskbuf[:, col0 + w * P:col0 + w * P + n] = m

    per_core = []
    for b in range(BSZ):
        prim, (oa_s, oa_e, oa_sc), (ob_s, ob_e, ob_sc) = core_data[b]
        maskbuf = np.zeros((P, NCH * P), np.float32)
        scatter = []
        for kb in range(NB):
            ps, pe, psc = prim[kb]
            c0 = min(kb, NB - 2)
            fill(maskbuf, _mask_chunk(kb, 0, FA) * P, 2, c0, ps, pe)
            scatter.extend((kb, j, ss, qi) for j, (ss, qi) in enumerate(psc))
        for a in range(FA):
            sl = slice(a * P, (a + 1) * P)
            fill(maskbuf, _mask_chunk(NB + a, 0, FA) * P, NA_W, 0,
                 oa_s[sl], oa_e[sl])
            scatter.extend((NB + a, j, ss, qi)
                           for j, (ss, qi) in enumerate(oa_sc[sl]))
        for v in range(FB):
            sl = slice(v * P, (v + 1) * P)
            fill(maskbuf, _mask_chunk(NB + FA + v, 0, FA) * P, NB_W,
                 NB - NB_W, ob_s[sl], ob_e[sl])
            scatter.extend((NB + FA + v, j, ss, qi)
                           for j, (ss, qi) in enumerate(ob_sc[sl]))
        xt = enc_in[b].reshape(NB, P, KT, P).transpose(3, 0, 2, 1).reshape(
            P, NB * KT * P).astype(ml_dtypes.bfloat16)
        xss = [np.ascontiguousarray(
                   xt[:, _SIGMA[j] * KT * P:(_SIGMA[j] + 1) * KT * P])
               for j in range(NB)]
        per_core.append((None, xss, maskbuf.astype(ml_dtypes.float8_e4m3),
                         scatter))

    wtiled16 = wtiled.astype(ml_dtypes.bfloat16)
    w0a = np.ascontiguousarray(wtiled16[:, :WCOLS // 2])
    w0b = np.ascontiguousarray(wtiled16[:, WCOLS // 2:])
    in_maps = []
    for _, xss, mb, _ in per_core:
        m = {"w0a": w0a, "w0b": w0b, "maskbuf": mb}
        for j, xs in enumerate(xss):
            m[f"xs{j}"] = xs
        in_maps.append(m)
    return T, FA, FB, in_maps, per_core, logit_bias, use_bias


def kernel(**inputs):
    T, FA, FB, in_maps, per_core, logit_bias, use_bias = _prep(inputs)
    assert not use_bias, "nonzero proj_b not supported in v3 path"
    key = (FA, FB, logit_bias)
    if key not in _cache:
        _cache[key] = _build_program(FA, FB, logit_bias)
    nc = _cache[key]
    r = run_bass_kernel_spmd(nc, in_maps, core_ids=list(range(BSZ)),
                             trace=bool(int(os.environ.get("KTRACE", "0"))))
    res1 = np.zeros((Q, PD), np.float32)
    res2 = np.zeros((Q, PD), np.float32)
    outs = (res1, res2)
    forder = _finish_order(FA, FB)
    slot_of = np.zeros(T, np.int64)
    for s, tid in enumerate(forder):
        slot_of[tid] = s
    NS = 6
    for b in range(BSZ):
        rb3 = np.asarray(r.results[b]["res3"], np.float32)  # [N3,128,3*257]
        rb1 = np.asarray(r.results[b]["res1"], np.float32)  # [NS,128,257]
        n3 = T - NS
        rb = np.concatenate([
            rb3.reshape(-1, P, 3, NOUT).transpose(0, 2, 1, 3).reshape(
                -1, P, NOUT)[:n3],
            rb1], axis=0)                                   # [T, 128, 257]
        scatter = per_core[b][3]
        ti = slot_of[np.array([x[0] for x in scatter])]
        jj = np.array([x[1] for x in scatter])
        ss = np.array([x[2] for x in scatter])
        qi = np.array([x[3] for x in scatter])
        num = rb[ti, jj]                                   # [n, 257]
        vals = num[:, :PD] / num[:, PD:PD + 1]
        for s in (0, 1):
            m = ss == s
            outs[s][qi[m]] = vals[m]
    kernel.last_exec_ns = r.exec_time_ns
    return res1, res2


# revision 64
# speedup vs baseline: 1.0098x; 1.0098x over previous
"""Span-attention kernel for Trainium2 (8 NeuronCores, SPMD).

Strategy (v4)
-------------
Data-parallel over bsz: core b owns batch row b (bsz == 8 == n_cores).
Host routes each query q to core qb[q]; both span sets are pooled
(the mask depends only on (start, end)) and bucketed by start>>7.
Each of the 16 buckets gets ONE primary query tile (128 slots) with a
2-chunk token window; per-core overflow goes to ovfA tiles (starts in
[0,1024), window chunks 0..8) and ovfB tiles (starts in [1024,2048),
window chunks 8..15).

Device traffic is minimized and DMA-friendly:
  * x and wext are pre-tiled on the host into the exact SBUF layout
    ([128 part, 128x128 k-tiles], bf16), one fully-contiguous DRAM
    param per load DMA.
  * masks are host-built dense fp8 {0,1} tiles in [token, query] lhsT
    layout -- no on-device mask construction.
  * output is the unnormalized [num | den] (bf16); the host divides.

Schedule: 12 dummy matmuls on a zero tile bridge the DMA ramp so the
PE HAM clock is un-throttled when real work starts; W is split across
both HWDGE rings ahead of the x slabs; the x feed runs on the sync
ring in consumption order; masks stream on the gpsimd ring in two
pieces (the late piece gated behind the x feed).  enc chunks are
produced in rotated order [1..15, 0] and phase-2 matmuls trail RUNWAY
iterations behind so mask-waiting work never head-of-line blocks the
PE queue; only primary 0 + ovfA remain after the last enc chunk.
Results stage as 4 three-tile groups (scalar-ring DMAs) + 6 singles
with dedicated staging tiles, the last finishers fanned across all
three rings.

Per-core device program:
  1. enc[2048, 257] = X_b @ [W | W@attn_w]  (PE bf16, 8 k-tiles per
     128-token chunk); ACT: E = exp(logit+bias); DVE scales the 256
     value cols by E into bf16 EncE; ACT writes E into col 256.
  2. out_ps[q, 0:257] = sum_w mask[w]^T @ EncE[chunk(w)]  (PE, fp8
     lhsT x bf16 rhs); overflow tiles accumulate one chunk per
     iteration so the tail is one matmul deep.
  3. DVE/ACT copy out_ps -> bf16 staging -> DMA out.
"""

import os
import sys

import numpy as np
import ml_dtypes

sys.path.insert(0, "/opt/trn_rl_repo")

from contextlib import ExitStack

from concourse import bass, bacc, mybir
import concourse.tile as tile
from concourse.bass_utils import run_bass_kernel_spmd

P = 128
BSZ = 8
SEQ = 2048
HD = 1024
PD = 256
NCOL = PD + 1   # value cols + logit col
NOUT = PD + 1   # value cols + denominator col
NB = SEQ // P   # 16 buckets
KT = HD // P    # 8 contraction tiles
Q = 8192
WCOLS = KT * NCOL          # 2056 pre-tiled wext columns
XOFF = WCOLS               # x tiles start here in the fused buffer
NA_W = 9                   # ovfA window chunks (0..8)
NB_W = 8                   # ovfB window chunks (8..15)

_cache = {}


_SIGMA = list(range(1, NB)) + [0]   # enc chunk production order


def _finish_order(FA, FB):
    """Tile ids in device finish order (res slot s holds the s-th
    finisher).  Rotated production [1..15, 0] means primary 0 and ovfA
    (whose windows contain chunk 0) finish last."""
    order = list(range(1, NB - 2))                    # primaries 1..13
    order += [NB + FA + b for b in range(FB)]         # ovfB (stop chunk 15)
    order += [NB - 2, NB - 1]                         # primaries 14, 15
    order += [NB + a for a in range(FA)]              # ovfA (stop chunk 0)
    order += [0]                                      # primary 0
    return order


def _mask_chunk(i, w, FA):
    """Chunk index in maskbuf for (tile i, window-pos w).
    Order: primaries 0..7, ovfA tiles, primaries 8..15, ovfB tiles."""
    if i < NB // 2:
        return i * 2 + w
    if NB <= i < NB + FA:
        return NB + (i - NB) * NA_W + w
    if i < NB:
        return NB + FA * NA_W + (i - NB // 2) * 2 + w
    return NB + FA * NA_W + NB + (i - NB - FA) * NB_W + w


def _build_program(FA, FB, logit_bias=0.0):
    T = NB + FA + FB
    NCH = NB * 2 + FA * NA_W + FB * NB_W
    NG = (T + 2) // 3
    nc = bacc.Bacc("TRN2", target_bir_lowering=False)
    f32 = mybir.dt.float32
    bf16 = mybir.dt.bfloat16
    fp8 = mybir.dt.float8e4

    # separate params per load slab: each DMA reads a fully-contiguous
    # DRAM region (strided column slices of one big buffer measured only
    # ~240 GB/s)
    w0a = nc.declare_dram_parameter("w0a", [P, WCOLS // 2], bf16,
                                    isOutput=False)
    w0b = nc.declare_dram_parameter("w0b", [P, WCOLS // 2], bf16,
                                    isOutput=False)
    xslabs = [nc.declare_dram_parameter(f"xs{j}", [P, KT * P], bf16,
                                        isOutput=False) for j in range(NB)]
    maskbuf = nc.declare_dram_parameter("maskbuf", [P, NCH * P], fp8,
                                        isOutput=False)
    # res groups: threes for the bulk (own param: contiguous 1.5 KiB
    # descriptor runs), singles for the last six finishers so the tail
    # drains in parallel across rings
    NS = 6
    groups = []
    for a in range(0, T - NS, 3):
        groups.append((a, min(a + 3, T - NS)))
    N3 = len(groups)
    groups += [(T - NS + s, T - NS + s + 1) for s in range(NS)]
    tile2group = {}
    for gi, (a, b) in enumerate(groups):
        for s in range(a, b):
            tile2group[s] = gi
    res3 = nc.declare_dram_parameter("res3", [N3, P, 3 * NOUT], bf16,
                                     isOutput=True)
    res1 = nc.declare_dram_parameter("res1", [NS, P, NOUT], bf16,
                                     isOutput=True)

    def mcol(i, w):
        return _mask_chunk(i, w, FA) * P

    MASK_SPLIT = (NB + FA * NA_W) * P   # end of the early piece

    with tile.TileContext(nc) as tc, ExitStack() as ctx:
        xw_pool = ctx.enter_context(tc.tile_pool(name="xw", bufs=1))
        mask_pool = ctx.enter_context(tc.tile_pool(name="mask", bufs=1))
        ecol_pool = ctx.enter_context(tc.tile_pool(name="ecol", bufs=1))
        ence_pool = ctx.enter_context(tc.tile_pool(name="ence", bufs=1))
        out_pool = ctx.enter_context(tc.tile_pool(name="out", bufs=3))
        single_pool = ctx.enter_context(tc.tile_pool(name="single", bufs=1))
        # PSUM budget: enc (shared with warmup) + out + (FA+FB ovf) <= 8
        spare = max(0, FA + FB - 2)
        ps_enc = ctx.enter_context(tc.tile_pool(name="ps_enc", bufs=max(2, 3 - spare), space="PSUM"))
        ps_out = ctx.enter_context(tc.tile_pool(name="ps_out", bufs=3, space="PSUM"))
        ps_ovf = ctx.enter_context(tc.tile_pool(name="ps_ovf", bufs=1, space="PSUM"))

        # ---- PE warmup: dummy matmuls bridge the DMA ramp so HAM is
        # un-throttled before the first real matmul ----
        warm_pool = ctx.enter_context(tc.tile_pool(name="warm", bufs=1))
        warm_sb = warm_pool.tile([P, 512], bf16, tag="warm_sb")
        nc.gpsimd.memset(warm_sb[:], 0.0)
        warm_ps = ps_enc.tile([P, 512], f32, tag="enc")
        for _ in range(12):
            nc.tensor.matmul(warm_ps[:], lhsT=warm_sb[:, 0:P],
                             rhs=warm_sb[:], start=True, stop=True,
                             skip_group_check=True)
        # keep the warmup alive past DCE: its result feeds a dram output
        warm_out = nc.declare_dram_parameter("warm_out", [1, 1], f32,
                                             isOutput=True)
        warm_res = warm_pool.tile([1, 1], f32, tag="warm_res")
        nc.vector.tensor_copy(warm_res[:], warm_ps[0:1, 0:1])
        nc.scalar.dma_start(warm_out[:], warm_res[:])

        # ---- loads: w on the scalar ring in parallel with the x feed on
        # the sync ring; masks on the gpsimd ring ----
        xw_sb = xw_pool.tile([P, WCOLS + NB * KT * P], bf16, tag="xw_sb")
        mask_sb = mask_pool.tile([P, NCH * P], fp8, tag="mask_sb")
        # W split across both HWDGE rings: k0-3 first on sync (ahead of the
        # x slabs), k4-7 on scalar -- the m0 k<4 matmuls can start as soon
        # as w0a+m0 land
        wa = nc.sync.dma_start(xw_sb[:, 0:WCOLS // 2], w0a[:])
        nc.scalar.dma_start(xw_sb[:, WCOLS // 2:WCOLS], w0b[:])
        nc.gpsimd.dma_start(mask_sb[:, 0:MASK_SPLIT], maskbuf[:, 0:MASK_SPLIT])
        slab_dmas = []
        for j in range(NB):
            c0 = WCOLS + j * KT * P
            # slab 1 rides the scalar ring right behind w0b so the PE never
            # starves between m0 and m1
            eng = nc.scalar if j == 1 else nc.sync
            slab_dmas.append(
                eng.dma_start(xw_sb[:, c0:c0 + KT * P], xslabs[j][:]))
        m2 = nc.gpsimd.dma_start(mask_sb[:, MASK_SPLIT:NCH * P],
                                 maskbuf[:, MASK_SPLIT:NCH * P])
        # late mask piece only needed from primary-8 emit; let x stream first
        bass._add_dep_helper(m2.ins, slab_dmas[6].ins, sync=True,
                             reason="stage late masks behind x feed")

        w_tiles = [xw_sb[:, k * NCOL:(k + 1) * NCOL] for k in range(KT)]
        enc_tiles = [None] * NB
        ovfA_ps = [None] * FA
        ovfB_ps = [None] * FB
        res_group = {}   # g -> [staging tile, n_written]
        slot_ctr = [0]

        def finish_tile(out_ps):
            slot = slot_ctr[0]
            slot_ctr[0] += 1
            g = tile2group[slot]
            a, bnd = groups[g]
            h = slot - a
            if g not in res_group:
                if g < N3:
                    rt = out_pool.tile([P, (bnd - a) * NOUT], bf16, tag="res")
                else:
                    # dedicated staging per single: the tail must never wait
                    # on an earlier group's DMA to free a pool slot
                    rt = single_pool.tile([P, NOUT], bf16, tag=f"ress{g - N3}")
                res_group[g] = [rt, 0]
            rg = res_group[g]
            use_dve = slot == T - 1 if slot >= T - 2 else slot % 2 == 0
            if use_dve:
                nc.vector.tensor_copy(rg[0][:, h * NOUT:(h + 1) * NOUT],
                                      out_ps[:])
            else:
                nc.scalar.activation(rg[0][:, h * NOUT:(h + 1) * NOUT],
                                     out_ps[:],
                                     mybir.ActivationFunctionType.Copy)
            rg[1] += 1
            if rg[1] == bnd - a:
                if g < N3:
                    n = bnd - a
                    nc.scalar.dma_start(res3[g][:, 0:n * NOUT], rg[0][:])
                else:
                    # HWDGE rings only: a tail DMA on the SWDGE ring makes
                    # the final GpSimd drain ~3.7us
                    eng = (nc.scalar, nc.sync, nc.sync,
                           nc.scalar, nc.scalar, nc.sync)[g - N3]
                    eng.dma_start(res1[a - (T - NS)], rg[0][:])

        def emit_primary(i):
            cs = [min(i, NB - 2), min(i, NB - 2) + 1]
            out_ps = ps_out.tile([P, NOUT], f32, tag="out")
            for w, c in enumerate(cs):
                nc.tensor.matmul(out_ps[:],
                                 lhsT=mask_sb[:, mcol(i, w):mcol(i, w) + P],
                                 rhs=enc_tiles[c][:],
                                 start=(w == 0), stop=(w == 1))
            finish_tile(out_ps)

        # phase-2 work for "virtual iteration" m, delayed RUNWAY iterations
        # behind enc production so mask-waiting matmuls never head-of-line
        # block the PE queue while the mask DMA is still in flight
        RUNWAY = 3

        def step(t):
            tpos = t - RUNWAY
            if not 0 <= tpos <= NB - 1:
                return
            c = _SIGMA[tpos]
            if c <= NA_W - 1:
                # ovfA accumulates in arrival order: chunk 1 first (tpos 0),
                # chunk 0 last (tpos NB-1)
                for a in range(FA):
                    if tpos == 0:
                        ova_tile = ps_ovf.tile([P, NOUT], f32, tag=f"ovA{a}")
                        ovfA_ps[a] = ova_tile
                    nc.tensor.matmul(
                        ovfA_ps[a][:],
                        lhsT=mask_sb[:, mcol(NB + a, c):mcol(NB + a, c) + P],
                        rhs=enc_tiles[c][:], start=(tpos == 0),
                        stop=(tpos == NB - 1), skip_group_check=True)
            if NB - NB_W <= c <= NB - 1:
                for b in range(FB):
                    w = c - (NB - NB_W)
                    if w == 0:
                        ovb_tile = ps_ovf.tile([P, NOUT], f32, tag=f"ovB{b}")
                        ovfB_ps[b] = ovb_tile
                    nc.tensor.matmul(
                        ovfB_ps[b][:],
                        lhsT=mask_sb[:, mcol(NB + FA + b, w):mcol(NB + FA + b, w) + P],
                        rhs=enc_tiles[c][:], start=(w == 0),
                        stop=(w == NB_W - 1), skip_group_check=True)
            # finish order must match _finish_order()
            if tpos == NB - 2:
                for b in range(FB):
                    finish_tile(ovfB_ps[b])
            if 1 <= tpos <= NB - 3:
                emit_primary(tpos)
            if tpos == NB - 2:
                emit_primary(NB - 2)
                emit_primary(NB - 1)
            if tpos == NB - 1:
                for a in range(FA):
                    finish_tile(ovfA_ps[a])
                emit_primary(0)

        # ---- phase 1 with interleaved (delayed) phase 2, enc chunks
        # produced in _SIGMA order ----
        for t in range(NB):
            m = _SIGMA[t]
            enc_ps = ps_enc.tile([P, NCOL], f32, tag="enc")
            xbase = XOFF + t * KT * P
            for k in range(KT):
                nc.tensor.matmul(
                    enc_ps[:], lhsT=xw_sb[:, xbase + k * P:xbase + (k + 1) * P],
                    rhs=w_tiles[k], start=(k == 0), stop=(k == KT - 1))
            ecol = ecol_pool.tile([P, 1], f32, tag=f"ecol{m}")
            nc.scalar.activation(ecol[:], enc_ps[:, PD:PD + 1],
                                 mybir.ActivationFunctionType.Exp,
                                 bias=float(logit_bias))
            ence = ence_pool.tile([P, NOUT], bf16, tag=f"ence{m}")
            nc.vector.tensor_scalar_mul(ence[:, 0:PD], enc_ps[:, 0:PD], ecol[:])
            nc.scalar.activation(ence[:, PD:PD + 1], ecol[:],
                                 mybir.ActivationFunctionType.Copy)
            enc_tiles[m] = ence
            step(t)
        for t in range(NB, NB + RUNWAY):
            step(t)

    nc.compile()
    return nc


def _prep(inputs):
    enc_in = np.asarray(inputs["encoded_input"], np.float32)
    proj_w = np.asarray(inputs["proj_w"], np.float32)
    proj_b = np.asarray(inputs["proj_b"], np.float32)
    attn_w = np.asarray(inputs["attn_w"], np.float32)
    attn_b = np.float32(np.asarray(inputs["attn_b"], np.float32))
    qb = np.asarray(inputs["query_batch_idx"], np.int64)
    s_all = [np.asarray(inputs["start_ids_1"], np.int64),
             np.asarray(inputs["start_ids_2"], np.int64)]
    e_all = [np.asarray(inputs["end_ids_1"], np.int64),
             np.asarray(inputs["end_ids_2"], np.int64)]

    waw = (proj_w @ attn_w)[:, None]
    wext = np.concatenate([proj_w, waw], axis=1)          # [HD, 257]
    logit_bias = float(proj_b @ attn_w + attn_b)
    use_bias = bool(np.any(proj_b != 0.0))
    wtiled = wext.reshape(KT, P, NCOL).transpose(1, 0, 2).reshape(P, WCOLS)

    tok = np.arange(P)
    # ---- bucket queries per core ----
    core_data = []
    FA = FB = 1
    for b in range(BSZ):
        sel = np.nonzero(qb == b)[0]
        prim = {kb: ([], [], []) for kb in range(NB)}
        oa_s, oa_e, oa_sc = [], [], []
        ob_s, ob_e, ob_sc = [], [], []
        for ss in range(2):
            s = s_all[ss][sel]
            e = e_all[ss][sel]
            kk = (s >> 7).astype(np.int64)
            for kb in range(NB):
                g = np.nonzero(kk == kb)[0]
                cur = prim[kb]
                room = P - len(cur[0])
                take, rest = g[:room], g[room:]
                cur[0].extend(s[take])
                cur[1].extend(e[take])
                cur[2].extend((ss, qi) for qi in sel[take])
                if len(rest):
                    if kb < NB // 2:
                        oa_s.extend(s[rest]); oa_e.extend(e[rest])
                        oa_sc.extend((ss, qi) for qi in sel[rest])
                    else:
                        ob_s.extend(s[rest]); ob_e.extend(e[rest])
                        ob_sc.extend((ss, qi) for qi in sel[rest])
        core_data.append((prim, (oa_s, oa_e, oa_sc), (ob_s, ob_e, ob_sc)))
        FA = max(FA, (len(oa_s) + P - 1) // P)
        FB = max(FB, (len(ob_s) + P - 1) // P)

    T = NB + FA + FB
    NCH = NB * 2 + FA * NA_W + FB * NB_W
    NG = (T + 2) // 3

    def fill(maskbuf, col0, nw, crow0, ss, ee):
        n = len(ss)
        if not n:
            return
        sa, ea = np.asarray(ss), np.asarray(ee)
        for w in range(nw):
            rows = tok + (crow0 + w) * P
            m = (rows[:, None] >= sa[None, :]) & (rows[:, None] <= ea[None, :])
            maskbuf[:, col0 + w * P:col0 + w * P + n] = m

    per_core = []
    for b in range(BSZ):
        prim, (oa_s, oa_e, oa_sc), (ob_s, ob_e, ob_sc) = core_data[b]
        maskbuf = np.zeros((P, NCH * P), np.float32)
        scatter = []
        for kb in range(NB):
            ps, pe, psc = prim[kb]
            c0 = min(kb, NB - 2)
            fill(maskbuf, _mask_chunk(kb, 0, FA) * P, 2, c0, ps, pe)
            scatter.extend((kb, j, ss, qi) for j, (ss, qi) in enumerate(psc))
        for a in range(FA):
            sl = slice(a * P, (a + 1) * P)
            fill(maskbuf, _mask_chunk(NB + a, 0, FA) * P, NA_W, 0,
                 oa_s[sl], oa_e[sl])
            scatter.extend((NB + a, j, ss, qi)
                           for j, (ss, qi) in enumerate(oa_sc[sl]))
        for v in range(FB):
            sl = slice(v * P, (v + 1) * P)
            fill(maskbuf, _mask_chunk(NB + FA + v, 0, FA) * P, NB_W,
                 NB - NB_W, ob_s[sl], ob_e[sl])
            scatter.extend((NB + FA + v, j, ss, qi)
                           for j, (ss, qi) in enumerate(ob_sc[sl]))
        xt = enc_in[b].reshape(NB, P, KT, P).transpose(3, 0, 2, 1).reshape(
            P, NB * KT * P).astype(ml_dtypes.bfloat16)
        xss = [np.ascontiguousarray(
                   xt[:, _SIGMA[j] * KT * P:(_SIGMA[j] + 1) * KT * P])
               for j in range(NB)]
        per_core.append((None, xss, maskbuf.astype(ml_dtypes.float8_e4m3),
                         scatter))

    wtiled16 = wtiled.astype(ml_dtypes.bfloat16)
    w0a = np.ascontiguousarray(wtiled16[:, :WCOLS // 2])
    w0b = np.ascontiguousarray(wtiled16[:, WCOLS // 2:])
    in_maps = []
    for _, xss, mb, _ in per_core:
        m = {"w0a": w0a, "w0b": w0b, "maskbuf": mb}
        for j, xs in enumerate(xss):
            m[f"xs{j}"] = xs
        in_maps.append(m)
    return T, FA, FB, in_maps, per_core, logit_bias, use_bias


def kernel(**inputs):
    T, FA, FB, in_maps, per_core, logit_bias, use_bias = _prep(inputs)
    assert not use_bias, "nonzero proj_b not supported in v3 path"
    key = (FA, FB, logit_bias)
    if key not in _cache:
        _cache[key] = _build_program(FA, FB, logit_bias)
    nc = _cache[key]
    r = run_bass_kernel_spmd(nc, in_maps, core_ids=list(range(BSZ)),
                             trace=bool(int(os.environ.get("KTRACE", "0"))))
    res1 = np.zeros((Q, PD), np.float32)
    res2 = np.zeros((Q, PD), np.float32)
    outs = (res1, res2)
    forder = _finish_order(FA, FB)
    slot_of = np.zeros(T, np.int64)
    for s, tid in enumerate(forder):
        slot_of[tid] = s
    NS = 6
    for b in range(BSZ):
        rb3 = np.asarray(r.results[b]["res3"], np.float32)  # [N3,128,3*257]
        rb1 = np.asarray(r.results[b]["res1"], np.float32)  # [NS,128,257]
        n3 = T - NS
        rb = np.concatenate([
            rb3.reshape(-1, P, 3, NOUT).transpose(0, 2, 1, 3).reshape(
                -1, P, NOUT)[:n3],
            rb1], axis=0)                                   # [T, 128, 257]
        scatter = per_core[b][3]
        ti = slot_of[np.array([x[0] for x in scatter])]
        jj = np.array([x[1] for x in scatter])
        ss = np.array([x[2] for x in scatter])
        qi = np.array([x[3] for x in scatter])
        num = rb[ti, jj]                                   # [n, 257]
        vals = num[:, :PD] / num[:, PD:PD + 1]
        for s in (0, 1):
            m = ss == s
            outs[s][qi[m]] = vals[m]
    kernel.last_exec_ns = r.exec_time_ns
    return res1, res2


# revision 67
# speedup vs baseline: 1.0396x; 1.0296x over previous
"""Span-attention kernel for Trainium2 (8 NeuronCores, SPMD).

Strategy (v4)
-------------
Data-parallel over bsz: core b owns batch row b (bsz == 8 == n_cores).
Host routes each query q to core qb[q]; both span sets are pooled
(the mask depends only on (start, end)) and bucketed by start>>7.
Each of the 16 buckets gets ONE primary query tile (128 slots) with a
2-chunk token window; per-core overflow goes to ovfA tiles (starts in
[0,1024), window chunks 0..8) and ovfB tiles (starts in [1024,2048),
window chunks 8..15).

Device traffic is minimized and DMA-friendly:
  * x and wext are pre-tiled on the host into the exact SBUF layout
    ([128 part, 128x128 k-tiles], bf16), one fully-contiguous DRAM
    param per load DMA.
  * masks are host-built dense fp8 {0,1} tiles in [token, query] lhsT
    layout -- no on-device mask construction.
  * output is the unnormalized [num | den] (bf16); the host divides.

Schedule: 12 dummy matmuls on a zero tile bridge the DMA ramp so the
PE HAM clock is un-throttled when real work starts; W is split across
both HWDGE rings ahead of the x slabs; the x feed runs on the sync
ring in consumption order; masks stream on the gpsimd ring in two
pieces (the late piece gated behind the x feed).  enc chunks are
produced in rotated order [1..15, 0] and phase-2 matmuls trail RUNWAY
iterations behind so mask-waiting work never head-of-line blocks the
PE queue; only primary 0 + ovfA remain after the last enc chunk.
Results stage as 4 three-tile groups (scalar-ring DMAs) + 6 singles
with dedicated staging tiles, the last finishers fanned across all
three rings.

Per-core device program:
  1. enc[2048, 257] = X_b @ [W | W@attn_w]  (PE bf16, 8 k-tiles per
     128-token chunk); ACT: E = exp(logit+bias); DVE scales the 256
     value cols by E into bf16 EncE; ACT writes E into col 256.
  2. out_ps[q, 0:257] = sum_w mask[w]^T @ EncE[chunk(w)]  (PE, fp8
     lhsT x bf16 rhs); overflow tiles accumulate one chunk per
     iteration so the tail is one matmul deep.
  3. DVE/ACT copy out_ps -> bf16 staging -> DMA out.
"""

import os
import sys

import numpy as np
import ml_dtypes

sys.path.insert(0, "/opt/trn_rl_repo")

from contextlib import ExitStack

from concourse import bass, bacc, mybir
import concourse.tile as tile
from concourse.bass_utils import run_bass_kernel_spmd

P = 128
BSZ = 8
SEQ = 2048
HD = 1024
PD = 256
NCOL = PD + 1   # value cols + logit col
NOUT = PD + 1   # value cols + denominator col
NB = SEQ // P   # 16 buckets
KT = HD // P    # 8 contraction tiles
Q = 8192
WCOLS = KT * NCOL          # 2056 pre-tiled wext columns
XOFF = WCOLS               # x tiles start here in the fused buffer
NA_W = 9                   # ovfA window chunks (0..8)
NB_W = 8                   # ovfB window chunks (8..15)

_cache = {}


_SIGMA = list(range(1, NB)) + [0]   # enc chunk production order


def _finish_order(FA, FB):
    """Tile ids in device finish order (res slot s holds the s-th
    finisher).  Rotated production [1..15, 0] means primary 0 and ovfA
    (whose windows contain chunk 0) finish last."""
    order = list(range(1, NB - 2))                    # primaries 1..13
    order += [NB + FA + b for b in range(FB)]         # ovfB (stop chunk 15)
    order += [NB - 2, NB - 1]                         # primaries 14, 15
    order += [NB + a for a in range(FA)]              # ovfA (stop chunk 0)
    order += [0]                                      # primary 0
    return order


def _mask_chunk(i, w, FA):
    """Chunk index in maskbuf for (tile i, window-pos w).
    Order: primaries 0..7, ovfA tiles, primaries 8..15, ovfB tiles."""
    if i < NB // 2:
        return i * 2 + w
    if NB <= i < NB + FA:
        return NB + (i - NB) * NA_W + w
    if i < NB:
        return NB + FA * NA_W + (i - NB // 2) * 2 + w
    return NB + FA * NA_W + NB + (i - NB - FA) * NB_W + w


def _build_program(FA, FB, logit_bias=0.0):
    T = NB + FA + FB
    NCH = NB * 2 + FA * NA_W + FB * NB_W
    NG = (T + 2) // 3
    nc = bacc.Bacc("TRN2", target_bir_lowering=False)
    f32 = mybir.dt.float32
    bf16 = mybir.dt.bfloat16
    fp8 = mybir.dt.float8e4

    # separate params per load slab: each DMA reads a fully-contiguous
    # DRAM region (strided column slices of one big buffer measured only
    # ~240 GB/s)
    w0a = nc.declare_dram_parameter("w0a", [P, WCOLS // 2], bf16,
                                    isOutput=False)
    w0b = nc.declare_dram_parameter("w0b", [P, WCOLS // 2], bf16,
                                    isOutput=False)
    xslabs = [nc.declare_dram_parameter(f"xs{j}", [P, KT * P], bf16,
                                        isOutput=False) for j in range(NB)]
    maskbuf = nc.declare_dram_parameter("maskbuf", [P, NCH * P], fp8,
                                        isOutput=False)
    # res groups: threes for the bulk (own param: contiguous 1.5 KiB
    # descriptor runs), singles for the last six finishers so the tail
    # drains in parallel across rings
    NS = 3
    groups = []
    for a in range(0, T - NS, 3):
        groups.append((a, min(a + 3, T - NS)))
    N3 = len(groups)
    groups += [(T - NS + s, T - NS + s + 1) for s in range(NS)]
    tile2group = {}
    for gi, (a, b) in enumerate(groups):
        for s in range(a, b):
            tile2group[s] = gi
    res3 = nc.declare_dram_parameter("res3", [N3, P, 3 * NOUT], bf16,
                                     isOutput=True)
    res1 = nc.declare_dram_parameter("res1", [NS, P, NOUT], bf16,
                                     isOutput=True)

    def mcol(i, w):
        return _mask_chunk(i, w, FA) * P

    MASK_SPLIT = (NB + FA * NA_W) * P   # end of the early piece

    with tile.TileContext(nc) as tc, ExitStack() as ctx:
        xw_pool = ctx.enter_context(tc.tile_pool(name="xw", bufs=1))
        mask_pool = ctx.enter_context(tc.tile_pool(name="mask", bufs=1))
        ecol_pool = ctx.enter_context(tc.tile_pool(name="ecol", bufs=1))
        ence_pool = ctx.enter_context(tc.tile_pool(name="ence", bufs=1))
        out_pool = ctx.enter_context(tc.tile_pool(name="out", bufs=3))
        single_pool = ctx.enter_context(tc.tile_pool(name="single", bufs=1))
        # PSUM budget: enc (shared with warmup) + out + (FA+FB ovf) <= 8
        spare = max(0, FA + FB - 2)
        ps_enc = ctx.enter_context(tc.tile_pool(name="ps_enc", bufs=max(2, 3 - spare), space="PSUM"))
        ps_out = ctx.enter_context(tc.tile_pool(name="ps_out", bufs=3, space="PSUM"))
        ps_ovf = ctx.enter_context(tc.tile_pool(name="ps_ovf", bufs=1, space="PSUM"))

        # ---- PE warmup: dummy matmuls bridge the DMA ramp so HAM is
        # un-throttled before the first real matmul ----
        warm_pool = ctx.enter_context(tc.tile_pool(name="warm", bufs=1))
        warm_sb = warm_pool.tile([P, 512], bf16, tag="warm_sb")
        nc.gpsimd.memset(warm_sb[:], 0.0)
        warm_ps = ps_enc.tile([P, 512], f32, tag="enc")
        for _ in range(12):
            nc.tensor.matmul(warm_ps[:], lhsT=warm_sb[:, 0:P],
                             rhs=warm_sb[:], start=True, stop=True,
                             skip_group_check=True)
        # keep the warmup alive past DCE: its result feeds a dram output
        warm_out = nc.declare_dram_parameter("warm_out", [1, 1], f32,
                                             isOutput=True)
        warm_res = warm_pool.tile([1, 1], f32, tag="warm_res")
        nc.vector.tensor_copy(warm_res[:], warm_ps[0:1, 0:1])
        nc.scalar.dma_start(warm_out[:], warm_res[:])

        # ---- loads: w on the scalar ring in parallel with the x feed on
        # the sync ring; masks on the gpsimd ring ----
        xw_sb = xw_pool.tile([P, WCOLS + NB * KT * P], bf16, tag="xw_sb")
        mask_sb = mask_pool.tile([P, NCH * P], fp8, tag="mask_sb")
        # W split across both HWDGE rings: k0-3 first on sync (ahead of the
        # x slabs), k4-7 on scalar -- the m0 k<4 matmuls can start as soon
        # as w0a+m0 land
        wa = nc.sync.dma_start(xw_sb[:, 0:WCOLS // 2], w0a[:])
        nc.scalar.dma_start(xw_sb[:, WCOLS // 2:WCOLS], w0b[:])
        nc.gpsimd.dma_start(mask_sb[:, 0:MASK_SPLIT], maskbuf[:, 0:MASK_SPLIT])
        slab_dmas = []
        for j in range(NB):
            c0 = WCOLS + j * KT * P
            # slab 1 rides the scalar ring right behind w0b so the PE never
            # starves between m0 and m1
            eng = nc.scalar if j == 1 else nc.sync
            slab_dmas.append(
                eng.dma_start(xw_sb[:, c0:c0 + KT * P], xslabs[j][:]))
        m2 = nc.gpsimd.dma_start(mask_sb[:, MASK_SPLIT:NCH * P],
                                 maskbuf[:, MASK_SPLIT:NCH * P])
        # late mask piece only needed from primary-8 emit; let x stream first
        bass._add_dep_helper(m2.ins, slab_dmas[6].ins, sync=True,
                             reason="stage late masks behind x feed")

        w_tiles = [xw_sb[:, k * NCOL:(k + 1) * NCOL] for k in range(KT)]
        enc_tiles = [None] * NB
        ovfA_ps = [None] * FA
        ovfB_ps = [None] * FB
        res_group = {}   # g -> [staging tile, n_written]
        slot_ctr = [0]

        def finish_tile(out_ps):
            slot = slot_ctr[0]
            slot_ctr[0] += 1
            g = tile2group[slot]
            a, bnd = groups[g]
            h = slot - a
            if g not in res_group:
                if g < N3:
                    rt = out_pool.tile([P, (bnd - a) * NOUT], bf16, tag="res")
                else:
                    # dedicated staging per single: the tail must never wait
                    # on an earlier group's DMA to free a pool slot
                    rt = single_pool.tile([P, NOUT], bf16, tag=f"ress{g - N3}")
                res_group[g] = [rt, 0]
            rg = res_group[g]
            use_dve = slot == T - 1 if slot >= T - 2 else slot % 2 == 0
            if use_dve:
                nc.vector.tensor_copy(rg[0][:, h * NOUT:(h + 1) * NOUT],
                                      out_ps[:])
            else:
                nc.scalar.activation(rg[0][:, h * NOUT:(h + 1) * NOUT],
                                     out_ps[:],
                                     mybir.ActivationFunctionType.Copy)
            rg[1] += 1
            if rg[1] == bnd - a:
                if g < N3:
                    n = bnd - a
                    nc.scalar.dma_start(res3[g][:, 0:n * NOUT], rg[0][:])
                else:
                    # HWDGE rings only: a tail DMA on the SWDGE ring makes
                    # the final GpSimd drain ~3.7us
                    eng = (nc.sync, nc.scalar, nc.sync)[g - N3]
                    eng.dma_start(res1[a - (T - NS)], rg[0][:])

        def emit_primary(i):
            cs = [min(i, NB - 2), min(i, NB - 2) + 1]
            out_ps = ps_out.tile([P, NOUT], f32, tag="out")
            for w, c in enumerate(cs):
                nc.tensor.matmul(out_ps[:],
                                 lhsT=mask_sb[:, mcol(i, w):mcol(i, w) + P],
                                 rhs=enc_tiles[c][:],
                                 start=(w == 0), stop=(w == 1))
            finish_tile(out_ps)

        # phase-2 work for "virtual iteration" m, delayed RUNWAY iterations
        # behind enc production so mask-waiting matmuls never head-of-line
        # block the PE queue while the mask DMA is still in flight
        RUNWAY = 3

        def step(t):
            tpos = t - RUNWAY
            if not 0 <= tpos <= NB - 1:
                return
            c = _SIGMA[tpos]
            if c <= NA_W - 1:
                # ovfA accumulates in arrival order: chunk 1 first (tpos 0),
                # chunk 0 last (tpos NB-1)
                for a in range(FA):
                    if tpos == 0:
                        ova_tile = ps_ovf.tile([P, NOUT], f32, tag=f"ovA{a}")
                        ovfA_ps[a] = ova_tile
                    nc.tensor.matmul(
                        ovfA_ps[a][:],
                        lhsT=mask_sb[:, mcol(NB + a, c):mcol(NB + a, c) + P],
                        rhs=enc_tiles[c][:], start=(tpos == 0),
                        stop=(tpos == NB - 1), skip_group_check=True)
            if NB - NB_W <= c <= NB - 1:
                for b in range(FB):
                    w = c - (NB - NB_W)
                    if w == 0:
                        ovb_tile = ps_ovf.tile([P, NOUT], f32, tag=f"ovB{b}")
                        ovfB_ps[b] = ovb_tile
                    nc.tensor.matmul(
                        ovfB_ps[b][:],
                        lhsT=mask_sb[:, mcol(NB + FA + b, w):mcol(NB + FA + b, w) + P],
                        rhs=enc_tiles[c][:], start=(w == 0),
                        stop=(w == NB_W - 1), skip_group_check=True)
            # finish order must match _finish_order()
            if tpos == NB - 2:
                for b in range(FB):
                    finish_tile(ovfB_ps[b])
            if 1 <= tpos <= NB - 3:
                emit_primary(tpos)
            if tpos == NB - 2:
                emit_primary(NB - 2)
                emit_primary(NB - 1)
            if tpos == NB - 1:
                for a in range(FA):
                    finish_tile(ovfA_ps[a])
                emit_primary(0)

        # ---- phase 1 with interleaved (delayed) phase 2, enc chunks
        # produced in _SIGMA order ----
        for t in range(NB):
            m = _SIGMA[t]
            enc_ps = ps_enc.tile([P, NCOL], f32, tag="enc")
            xbase = XOFF + t * KT * P
            for k in range(KT):
                nc.tensor.matmul(
                    enc_ps[:], lhsT=xw_sb[:, xbase + k * P:xbase + (k + 1) * P],
                    rhs=w_tiles[k], start=(k == 0), stop=(k == KT - 1))
            ecol = ecol_pool.tile([P, 1], f32, tag=f"ecol{m}")
            nc.scalar.activation(ecol[:], enc_ps[:, PD:PD + 1],
                                 mybir.ActivationFunctionType.Exp,
                                 bias=float(logit_bias))
            ence = ence_pool.tile([P, NOUT], bf16, tag=f"ence{m}")
            nc.vector.tensor_scalar_mul(ence[:, 0:PD], enc_ps[:, 0:PD], ecol[:])
            nc.scalar.activation(ence[:, PD:PD + 1], ecol[:],
                                 mybir.ActivationFunctionType.Copy)
            enc_tiles[m] = ence
            step(t)
        for t in range(NB, NB + RUNWAY):
            step(t)

    nc.compile()
    return nc


def _prep(inputs):
    enc_in = np.asarray(inputs["encoded_input"], np.float32)
    proj_w = np.asarray(inputs["proj_w"], np.float32)
    proj_b = np.asarray(inputs["proj_b"], np.float32)
    attn_w = np.asarray(inputs["attn_w"], np.float32)
    attn_b = np.float32(np.asarray(inputs["attn_b"], np.float32))
    qb = np.asarray(inputs["query_batch_idx"], np.int64)
    s_all = [np.asarray(inputs["start_ids_1"], np.int64),
             np.asarray(inputs["start_ids_2"], np.int64)]
    e_all = [np.asarray(inputs["end_ids_1"], np.int64),
             np.asarray(inputs["end_ids_2"], np.int64)]

    waw = (proj_w @ attn_w)[:, None]
    wext = np.concatenate([proj_w, waw], axis=1)          # [HD, 257]
    logit_bias = float(proj_b @ attn_w + attn_b)
    use_bias = bool(np.any(proj_b != 0.0))
    wtiled = wext.reshape(KT, P, NCOL).transpose(1, 0, 2).reshape(P, WCOLS)

    tok = np.arange(P)
    # ---- bucket queries per core ----
    core_data = []
    FA = FB = 1
    for b in range(BSZ):
        sel = np.nonzero(qb == b)[0]
        prim = {kb: ([], [], []) for kb in range(NB)}
        oa_s, oa_e, oa_sc = [], [], []
        ob_s, ob_e, ob_sc = [], [], []
        for ss in range(2):
            s = s_all[ss][sel]
            e = e_all[ss][sel]
            kk = (s >> 7).astype(np.int64)
            for kb in range(NB):
                g = np.nonzero(kk == kb)[0]
                cur = prim[kb]
                room = P - len(cur[0])
                take, rest = g[:room], g[room:]
                cur[0].extend(s[take])
                cur[1].extend(e[take])
                cur[2].extend((ss, qi) for qi in sel[take])
                if len(rest):
                    if kb < NB // 2:
                        oa_s.extend(s[rest]); oa_e.extend(e[rest])
                        oa_sc.extend((ss, qi) for qi in sel[rest])
                    else:
                        ob_s.extend(s[rest]); ob_e.extend(e[rest])
                        ob_sc.extend((ss, qi) for qi in sel[rest])
        core_data.append((prim, (oa_s, oa_e, oa_sc), (ob_s, ob_e, ob_sc)))
        FA = max(FA, (len(oa_s) + P - 1) // P)
        FB = max(FB, (len(ob_s) + P - 1) // P)

    T = NB + FA + FB
    NCH = NB * 2 + FA * NA_W + FB * NB_W
    NG = (T + 2) // 3

    def fill(maskbuf, col0, nw, crow0, ss, ee):
        n = len(ss)
        if not n:
            return
        sa, ea = np.asarray(ss), np.asarray(ee)
        for w in range(nw):
            rows = tok + (crow0 + w) * P
            m = (rows[:, None] >= sa[None, :]) & (rows[:, None] <= ea[None, :])
            maskbuf[:, col0 + w * P:col0 + w * P + n] = m

    per_core = []
    for b in range(BSZ):
        prim, (oa_s, oa_e, oa_sc), (ob_s, ob_e, ob_sc) = core_data[b]
        maskbuf = np.zeros((P, NCH * P), np.float32)
        scatter = []
        for kb in range(NB):
            ps, pe, psc = prim[kb]
            c0 = min(kb, NB - 2)
            fill(maskbuf, _mask_chunk(kb, 0, FA) * P, 2, c0, ps, pe)
            scatter.extend((kb, j, ss, qi) for j, (ss, qi) in enumerate(psc))
        for a in range(FA):
            sl = slice(a * P, (a + 1) * P)
            fill(maskbuf, _mask_chunk(NB + a, 0, FA) * P, NA_W, 0,
                 oa_s[sl], oa_e[sl])
            scatter.extend((NB + a, j, ss, qi)
                           for j, (ss, qi) in enumerate(oa_sc[sl]))
        for v in range(FB):
            sl = slice(v * P, (v + 1) * P)
            fill(maskbuf, _mask_chunk(NB + FA + v, 0, FA) * P, NB_W,
                 NB - NB_W, ob_s[sl], ob_e[sl])
            scatter.extend((NB + FA + v, j, ss, qi)
                           for j, (ss, qi) in enumerate(ob_sc[sl]))
        xt = enc_in[b].reshape(NB, P, KT, P).transpose(3, 0, 2, 1).reshape(
            P, NB * KT * P).astype(ml_dtypes.bfloat16)
        xss = [np.ascontiguousarray(
                   xt[:, _SIGMA[j] * KT * P:(_SIGMA[j] + 1) * KT * P])
               for j in range(NB)]
        per_core.append((None, xss, maskbuf.astype(ml_dtypes.float8_e4m3),
                         scatter))

    wtiled16 = wtiled.astype(ml_dtypes.bfloat16)
    w0a = np.ascontiguousarray(wtiled16[:, :WCOLS // 2])
    w0b = np.ascontiguousarray(wtiled16[:, WCOLS // 2:])
    in_maps = []
    for _, xss, mb, _ in per_core:
        m = {"w0a": w0a, "w0b": w0b, "maskbuf": mb}
        for j, xs in enumerate(xss):
            m[f"xs{j}"] = xs
        in_maps.append(m)
    return T, FA, FB, in_maps, per_core, logit_bias, use_bias


def kernel(**inputs):
    T, FA, FB, in_maps, per_core, logit_bias, use_bias = _prep(inputs)
    assert not use_bias, "nonzero proj_b not supported in v3 path"
    key = (FA, FB, logit_bias)
    if key not in _cache:
        _cache[key] = _build_program(FA, FB, logit_bias)
    nc = _cache[key]
    r = run_bass_kernel_spmd(nc, in_maps, core_ids=list(range(BSZ)),
                             trace=bool(int(os.environ.get("KTRACE", "0"))))
    res1 = np.zeros((Q, PD), np.float32)
    res2 = np.zeros((Q, PD), np.float32)
    outs = (res1, res2)
    forder = _finish_order(FA, FB)
    slot_of = np.zeros(T, np.int64)
    for s, tid in enumerate(forder):
        slot_of[tid] = s
    NS = 3
    for b in range(BSZ):
        rb3 = np.asarray(r.results[b]["res3"], np.float32)  # [N3,128,3*257]
        rb1 = np.asarray(r.results[b]["res1"], np.float32)  # [NS,128,257]
        n3 = T - NS
        rb = np.concatenate([
            rb3.reshape(-1, P, 3, NOUT).transpose(0, 2, 1, 3).reshape(
                -1, P, NOUT)[:n3],
            rb1], axis=0)                                   # [T, 128, 257]
        scatter = per_core[b][3]
        ti = slot_of[np.array([x[0] for x in scatter])]
        jj = np.array([x[1] for x in scatter])
        ss = np.array([x[2] for x in scatter])
        qi = np.array([x[3] for x in scatter])
        num = rb[ti, jj]                                   # [n, 257]
        vals = num[:, :PD] / num[:, PD:PD + 1]
        for s in (0, 1):
            m = ss == s
            outs[s][qi[m]] = vals[m]
    kernel.last_exec_ns = r.exec_time_ns
    return res1, res2
